# revision 1
# baseline (speedup 1.0000x reference)
"""GATModel (2-layer GAT, N nodes, E edges + self-loops) on 8 Trainium2 NeuronCores.

Sharding: nodes and edges are sharded by destination-node owner (8 cores x
N/8 nodes). Per core, destination nodes are processed in 128-node windows;
edges of a window are padded into 128-edge chunks. Message gathers use
dma_gather (int16 indices -> the node table is split into an A half
[0, TSPLIT) and a B half [TSPLIT, N)); the segment softmax + scatter-add is
one-hot matmuls accumulated in PSUM (trailing columns carry the softmax
denominators). Layer-1 attention logits are linear in x and are precomputed
on host per edge. Between layers, each core's compact [h2|s2src|s2dst] rows
are AllGathered (4 overlapped chunks) and restrided into 512-byte rows for
the layer-2 gathers.
"""

import math

import numpy as np

import concourse.bacc as bacc
import concourse.bass as bass
import concourse.mybir as mybir
import concourse.tile as tile
from concourse.bass_utils import run_bass_kernel_spmd
from concourse.masks import make_identity

F32 = mybir.dt.float32
I16 = mybir.dt.int16
I32 = mybir.dt.int32
AF = mybir.ActivationFunctionType
OP = mybir.AluOpType
AX = mybir.AxisListType

NEG_SLOPE = 0.2


class Cfg:
    def __init__(self, n, in_dim, hid, heads1, out_dim, tsplit, g=3):
        self.N = n
        self.IN = in_dim          # 256
        self.C1 = hid             # 16
        self.H1 = heads1          # 8
        self.F1 = hid * heads1    # 128
        self.F2 = out_dim         # 64
        self.NC = 8
        assert n % self.NC == 0
        self.NLOC = n // self.NC
        self.WIN = 128
        self.NW = math.ceil(self.NLOC / self.WIN)
        self.LASTROWS = self.NLOC - (self.NW - 1) * self.WIN
        self.TSPLIT = tsplit
        self.G = g
        self.NB = math.ceil(self.NW / g)


FULL = Cfg(50000, 256, 16, 8, 64, 32768)


def _wrap_idx(ids):
    """int16 index list (len % 128 == 0) -> [128, L/16] dma_gather layout."""
    L = len(ids)
    assert L % 128 == 0
    w = ids.reshape(L // 16, 16).T
    return np.tile(w, (8, 1)).astype(np.int16)


def host_prep(c, x, edge_index, W1, a1_src, a1_dst, b1, W2, a2_src, a2_dst, b2):
    loop = np.arange(c.N, dtype=np.int64)
    src = np.concatenate([np.asarray(edge_index[0]), loop]).astype(np.int64)
    dst = np.concatenate([np.asarray(edge_index[1]), loop]).astype(np.int64)

    x = np.asarray(x, dtype=np.float32)
    W1 = np.asarray(W1, dtype=np.float32)
    W2 = np.asarray(W2, dtype=np.float32)

    # layer-1 logits are linear in x: s1_src = x @ (W1 . a1_src) etc.
    W1r = W1.reshape(c.IN, c.H1, c.C1)
    Wa1s = np.einsum("ihc,hc->ih", W1r, np.asarray(a1_src, np.float32))
    Wa1d = np.einsum("ihc,hc->ih", W1r, np.asarray(a1_dst, np.float32))
    ssum_e = (x @ Wa1s)[src] + (x @ Wa1d)[dst]  # [E, H1]

    owner = dst // c.NLOC
    KA = KB = 1
    percore = []
    for ci in range(c.NC):
        m = owner == ci
        s_c = src[m]
        dl_c = dst[m] - ci * c.NLOC
        ss_c = ssum_e[m]
        win = dl_c // c.WIN
        isB = s_c >= c.TSPLIT
        wins = []
        for w in range(c.NW):
            wm = win == w
            a_m = wm & ~isB
            b_m = wm & isB
            ea = (s_c[a_m], dl_c[a_m], ss_c[a_m])
            eb = (s_c[b_m] - c.TSPLIT, dl_c[b_m], ss_c[b_m])
            KA = max(KA, math.ceil(max(len(ea[0]), 1) / 128))
            KB = max(KB, math.ceil(max(len(eb[0]), 1) / 128))
            wins.append((ea, eb))
        percore.append(wins)

    c.KA, c.KB = KA, KB
    nk_w = KA + KB

    batches = []
    offC = offA = offB = 0
    for b in range(c.NB):
        nw = min(c.G, c.NW - b * c.G)
        batches.append((nw, offC, offA, offB))
        offC += nw * nk_w
        offA += nw * KA * 8
        offB += nw * KB * 8
    c.batches = batches
    c.TOTC, c.TOTA, c.TOTB = offC, offA, offB

    ins = []
    for ci in range(c.NC):
        wins = percore[ci]
        idxA = np.zeros((128, offA), np.int16)
        idxB = np.zeros((128, offB), np.int16)
        dstloc = np.full((128, offC), -1.0, np.float32)
        ssum = np.zeros((128, offC * c.H1), np.float32)
        dstid = np.zeros((128, offC), np.int32)

        for b, (nw, oc, oa, ob) in enumerate(batches):
            a_ids = np.zeros(nw * KA * 128, np.int16)
            b_ids = np.zeros(nw * KB * 128, np.int16)
            for wb in range(nw):
                w = b * c.G + wb
                for half, ids_arr, K, slot0, chunk0 in (
                    (0, a_ids, KA, wb * KA * 128, oc + wb * KA),
                    (1, b_ids, KB, wb * KB * 128, oc + nw * KA + wb * KB),
                ):
                    sH, dlH, ssH = wins[w][half]
                    n = len(sH)
                    assert n <= K * 128
                    ids_arr[slot0 : slot0 + n] = sH.astype(np.int16)
                    wcH = (dlH % c.WIN).astype(np.float32)
                    for jj in range(K):
                        lo, hi = jj * 128, min(n, (jj + 1) * 128)
                        if hi <= lo:
                            break
                        j = chunk0 + jj
                        dstloc[0 : hi - lo, j] = wcH[lo:hi]
                        dstid[0 : hi - lo, j] = dlH[lo:hi]
                        ssum[0 : hi - lo, j * c.H1 : (j + 1) * c.H1] = ssH[lo:hi]
            idxA[:, oa : oa + nw * KA * 8] = _wrap_idx(a_ids)
            idxB[:, ob : ob + nw * KB * 8] = _wrap_idx(b_ids)

        # s2dst gather indices: local dst node id per slot, in batch slot order
        idxD = np.zeros((128, offC * 8), np.int16)
        for b, (nw, oc, oa, ob) in enumerate(batches):
            nk = nw * nk_w
            ids = dstid[:, oc : oc + nk].T.reshape(-1).astype(np.int16)
            idxD[:, oc * 8 : (oc + nk) * 8] = _wrap_idx(ids)

        ins.append(dict(idxA=idxA, idxB=idxB, idxD=idxD, dstloc=dstloc, ssum=ssum))

    xT = np.ascontiguousarray(x.T)
    W2e = np.zeros((c.F1, 66), np.float32)
    W2e[:, : c.F2] = W2
    W2e[:, c.F2] = W2 @ np.asarray(a2_src, np.float32)[0]
    W2e[:, c.F2 + 1] = W2 @ np.asarray(a2_dst, np.float32)[0]
    b1t = np.tile(np.asarray(b1, np.float32)[None, :], (128, 1))
    b2t = np.tile(np.asarray(b2, np.float32)[None, :], (128, 1))
    for d in ins:
        d.update(xT=xT, W1=W1, W2e=W2e, b1t=b1t, b2t=b2t)
    return ins


def build_program(c, reps=1):
    nc = bacc.Bacc()
    f32 = F32
    H1, F1, F2, C1 = c.H1, c.F1, c.F2, c.C1
    KA, KB, G, WIN, NW = c.KA, c.KB, c.G, c.WIN, c.NW
    N1 = F1 + H1   # layer-1 rhs width (messages + denom cols)
    N2 = F2 + 1    # layer-2 rhs width

    xT = nc.dram_tensor("xT", [c.IN, c.N], f32, kind="ExternalInput")
    W1 = nc.dram_tensor("W1", [c.IN, F1], f32, kind="ExternalInput")
    W2e = nc.dram_tensor("W2e", [F1, 66], f32, kind="ExternalInput")
    b1t_d = nc.dram_tensor("b1t", [128, F1], f32, kind="ExternalInput")
    b2t_d = nc.dram_tensor("b2t", [128, F2], f32, kind="ExternalInput")
    idxA_d = nc.dram_tensor("idxA", [128, c.TOTA], I16, kind="ExternalInput")
    idxB_d = nc.dram_tensor("idxB", [128, c.TOTB], I16, kind="ExternalInput")
    dstloc_d = nc.dram_tensor("dstloc", [128, c.TOTC], f32, kind="ExternalInput")
    ssum_d = nc.dram_tensor("ssum", [128, c.TOTC * H1], f32, kind="ExternalInput")
    idxD_d = nc.dram_tensor("idxD", [128, c.TOTC * 8], I16, kind="ExternalInput")
    out_d = nc.dram_tensor("out", [c.NLOC, F2], f32, kind="ExternalOutput")

    table1 = nc.dram_tensor("table1", [c.N, F1], f32)
    table2 = nc.dram_tensor("table2", [c.N, 128], f32)
    agin = nc.dram_tensor("agin", [c.NLOC, 66], f32)
    # per-local-node s2dst, padded to 256B rows so dma_gather can fetch it
    s2dst_pad = nc.dram_tensor("s2dstpad", [c.NLOC, 64], f32)

    # AllGather chunks (by whole windows)
    nag = min(4, NW)
    per = math.ceil(NW / nag)
    wchunks = []
    w0 = 0
    while w0 < NW:
        wchunks.append((w0, min(w0 + per, NW)))
        w0 = min(w0 + per, NW)
    ag_rows = [(wl * WIN, min(wh * WIN, c.NLOC)) for wl, wh in wchunks]
    ag_out = [
        nc.dram_tensor(f"agout{k}", [c.NC * (rhi - rlo), 66], f32, addr_space="Shared")
        for k, (rlo, rhi) in enumerate(ag_rows)
    ]
    rg = [list(range(c.NC))]

    with tile.TileContext(nc) as tc:
        with (
            tc.tile_pool(name="const", bufs=1) as constp,
            tc.tile_pool(name="p1", bufs=3) as p1p,
            tc.tile_pool(name="meta", bufs=2) as metap,
            tc.tile_pool(name="big", bufs=2) as bigp,
            tc.tile_pool(name="schunk", bufs=6) as sp,
            tc.tile_pool(name="win", bufs=3) as winp,
            tc.tile_pool(name="pw", bufs=3, space="PSUM") as pwp,
            tc.tile_pool(name="ph", bufs=2, space="PSUM") as php,
            tc.tile_pool(name="ptp", bufs=2, space="PSUM") as ptp,
            tc.tile_pool(name="pp2", bufs=1, space="PSUM") as pp2,
        ):
            # ---- constants ----
            ident = constp.tile([128, 128], f32)
            make_identity(nc, ident[:])
            iota_i = constp.tile([128, 128], I32)
            nc.gpsimd.iota(iota_i[:], pattern=[[1, 128]], base=0, channel_multiplier=0)
            iotaf = constp.tile([128, 128], f32)
            nc.vector.tensor_copy(iotaf[:], iota_i[:])
            w1a = constp.tile([128, F1], f32)
            nc.sync.dma_start(w1a[:], W1[0:128, :])
            w1b = constp.tile([128, F1], f32)
            nc.sync.dma_start(w1b[:], W1[128:256, :])
            w2t = constp.tile([128, 66], f32)
            nc.sync.dma_start(w2t[:], W2e[:, :])
            b1s = constp.tile([128, F1], f32)
            nc.sync.dma_start(b1s[:], b1t_d[:, :])
            b2s = constp.tile([128, F2], f32)
            nc.sync.dma_start(b2s[:], b2t_d[:, :])

            for _rep in range(reps):
                # ---- P1: table1 = x @ W1 for all nodes (replicated) ----
                for t in range(math.ceil(c.N / 128)):
                    r0 = t * 128
                    rows = min(128, c.N - r0)
                    xa = p1p.tile([128, 128], f32, tag="xa")
                    nc.sync.dma_start(xa[:, :rows], xT[0:128, r0 : r0 + rows])
                    xb = p1p.tile([128, 128], f32, tag="xb")
                    nc.sync.dma_start(xb[:, :rows], xT[128:256, r0 : r0 + rows])
                    ph = php.tile([128, F1], f32, tag="ph")
                    nc.tensor.matmul(ph[:rows, :], lhsT=xa[:, :rows], rhs=w1a[:], start=True, stop=False)
                    nc.tensor.matmul(ph[:rows, :], lhsT=xb[:, :rows], rhs=w1b[:], start=False, stop=True)
                    hs = p1p.tile([128, F1], f32, tag="hs")
                    nc.vector.tensor_copy(hs[:rows, :], ph[:rows, :])
                    nc.sync.dma_start(table1[r0 : r0 + rows, :], hs[:rows, :])

                # ---- P2 (layer-1 edges) + fused P3 + chunked AllGather ----
                ag_issued = 0
                for b, (nw, oc, oa, ob) in enumerate(c.batches):
                    nA, nB_ = nw * KA, nw * KB
                    nk = nA + nB_
                    idxa_t = metap.tile([128, nA * 8], I16, tag="idxa")
                    nc.sync.dma_start(idxa_t[:], idxA_d[:, oa : oa + nA * 8])
                    idxb_t = metap.tile([128, nB_ * 8], I16, tag="idxb")
                    nc.sync.dma_start(idxb_t[:], idxB_d[:, ob : ob + nB_ * 8])
                    dl_t = metap.tile([128, nk], f32, tag="dl")
                    nc.sync.dma_start(dl_t[:], dstloc_d[:, oc : oc + nk])
                    ss_t = metap.tile([128, nk * H1], f32, tag="ss")
                    nc.sync.dma_start(ss_t[:], ssum_d[:, oc * H1 : (oc + nk) * H1])

                    M1 = bigp.tile([128, nk, 128], f32, tag="M")
                    nc.gpsimd.dma_gather(
                        M1[:, 0:nA, :], table1[0 : c.TSPLIT, :], idxa_t[:],
                        nA * 128, nA * 128, F1, single_packet=False,
                    )
                    nc.gpsimd.dma_gather(
                        M1[:, nA:nk, :], table1[c.TSPLIT : c.N, :], idxb_t[:],
                        nB_ * 128, nB_ * 128, F1, single_packet=False,
                    )

                    rhs = bigp.tile([128, nk, N1], f32, tag="rhs")
                    lr = metap.tile([128, nk * H1], f32, tag="lr")
                    nc.vector.tensor_scalar_mul(lr[:], ss_t[:], NEG_SLOPE)
                    nc.vector.tensor_tensor(lr[:], lr[:], ss_t[:], OP.max)
                    nc.scalar.activation(
                        rhs[:, :, F1:N1],
                        lr[:].rearrange("p (k h) -> p k h", h=H1),
                        AF.Exp,
                    )
                    nc.vector.tensor_tensor(
                        out=rhs[:, :, 0:F1].rearrange("p k (h ch) -> p k h ch", h=H1),
                        in0=M1[:, :, :].rearrange("p k (h ch) -> p k h ch", h=H1),
                        in1=rhs[:, :, F1:N1][:, :, :, None].to_broadcast([128, nk, H1, C1]),
                        op=OP.mult,
                    )

                    for wb in range(nw):
                        w = b * G + wb
                        rows = WIN if w < NW - 1 else c.LASTROWS
                        pw = pwp.tile([128, N1], f32, tag="pw")
                        cl = [wb * KA + i for i in range(KA)] + [
                            nA + wb * KB + i for i in range(KB)
                        ]
                        for cidx, j in enumerate(cl):
                            S = sp.tile([128, 128], f32, tag="S")
                            nc.vector.tensor_tensor(
                                S[:], dl_t[:, j : j + 1].to_broadcast([128, 128]),
                                iotaf[:], OP.is_equal,
                            )
                            nc.tensor.matmul(
                                pw[:], lhsT=S[:], rhs=rhs[:, j, :],
                                start=(cidx == 0), stop=(cidx == len(cl) - 1),
                            )
                        # window epilogue: normalize, +b1, ELU
                        rcp = winp.tile([128, H1], f32, tag="rcp")
                        nc.vector.reciprocal(rcp[:], pw[:, F1:N1])
                        h1p = winp.tile([128, F1], f32, tag="h1p")
                        nc.vector.tensor_tensor(
                            h1p[:].rearrange("p (h ch) -> p h ch", h=H1),
                            pw[:, 0:F1].rearrange("p (h ch) -> p h ch", h=H1),
                            rcp[:, :, None].to_broadcast([128, H1, C1]),
                            OP.mult,
                        )
                        nc.vector.tensor_add(h1p[:], h1p[:], b1s[:])
                        em = winp.tile([128, F1], f32, tag="em")
                        nc.vector.tensor_scalar_min(em[:], h1p[:], 0.0)
                        nc.scalar.activation(em[:], em[:], AF.Exp)
                        nc.vector.tensor_scalar_max(h1p[:], h1p[:], 0.0)
                        nc.vector.tensor_add(h1p[:], h1p[:], em[:])
                        nc.vector.tensor_scalar_add(h1p[:], h1p[:], -1.0)
                        # fused P3: [h2 | s2src | s2dst] for this window
                        tp = ptp.tile([128, 128], f32, tag="tp")
                        nc.tensor.transpose(tp[:], h1p[:], ident[:])
                        h1T = winp.tile([128, 128], f32, tag="h1T")
                        nc.vector.tensor_copy(h1T[:], tp[:])
                        p2 = pp2.tile([128, 66], f32, tag="p2")
                        nc.tensor.matmul(
                            p2[:rows, :], lhsT=h1T[:, :rows], rhs=w2t[:], start=True, stop=True
                        )
                        t2 = winp.tile([128, 66], f32, tag="t2")
                        nc.vector.tensor_copy(t2[:rows, :], p2[:rows, :])
                        nc.sync.dma_start(agin[w * WIN : w * WIN + rows, :], t2[:rows, :])
                        nc.sync.dma_start(
                            s2dst_pad[w * WIN : w * WIN + rows, 0:1], t2[:rows, 65:66]
                        )

                    while ag_issued < len(wchunks) and wchunks[ag_issued][1] <= (b + 1) * G:
                        k = ag_issued
                        rlo, rhi = ag_rows[k]
                        nc.gpsimd.collective_compute(
                            "AllGather", OP.bypass, replica_groups=rg,
                            ins=[agin[rlo:rhi, :]], outs=[ag_out[k][:, :]],
                        )
                        rk = rhi - rlo
                        for r in range(c.NC):
                            nc.sync.dma_start(
                                table2[r * c.NLOC + rlo : r * c.NLOC + rhi, 0:66],
                                ag_out[k][r * rk : (r + 1) * rk, :],
                            )
                        ag_issued += 1

                # ---- P4: layer-2 edges ----
                for b, (nw, oc, oa, ob) in enumerate(c.batches):
                    nA, nB_ = nw * KA, nw * KB
                    nk = nA + nB_
                    idxa_t = metap.tile([128, nA * 8], I16, tag="idxa")
                    nc.sync.dma_start(idxa_t[:], idxA_d[:, oa : oa + nA * 8])
                    idxb_t = metap.tile([128, nB_ * 8], I16, tag="idxb")
                    nc.sync.dma_start(idxb_t[:], idxB_d[:, ob : ob + nB_ * 8])
                    dl_t = metap.tile([128, nk], f32, tag="dl")
                    nc.sync.dma_start(dl_t[:], dstloc_d[:, oc : oc + nk])
                    idxd_t = metap.tile([128, nk * 8], I16, tag="idxd")
                    nc.sync.dma_start(idxd_t[:], idxD_d[:, oc * 8 : (oc + nk) * 8])

                    M2 = bigp.tile([128, nk, 128], f32, tag="M")
                    nc.gpsimd.dma_gather(
                        M2[:, 0:nA, :], table2[0 : c.TSPLIT, :], idxa_t[:],
                        nA * 128, nA * 128, 128, single_packet=False,
                    )
                    nc.gpsimd.dma_gather(
                        M2[:, nA:nk, :], table2[c.TSPLIT : c.N, :], idxb_t[:],
                        nB_ * 128, nB_ * 128, 128, single_packet=False,
                    )
                    SD = bigp.tile([128, nk, 64], f32, tag="SD")
                    nc.gpsimd.dma_gather(
                        SD[:, :, :], s2dst_pad[:, :], idxd_t[:], nk * 128, nk * 128, 64, single_packet=False
                    )
                    sc = metap.tile([128, nk], f32, tag="sc")
                    nc.vector.tensor_tensor(sc[:], M2[:, :, F2], SD[:, :, 0], OP.add)
                    sc2 = metap.tile([128, nk], f32, tag="sc2")
                    nc.vector.tensor_scalar_mul(sc2[:], sc[:], NEG_SLOPE)
                    nc.vector.tensor_tensor(sc[:], sc[:], sc2[:], OP.max)
                    rhs2 = bigp.tile([128, nk, N2], f32, tag="rhs")
                    nc.scalar.activation(rhs2[:, :, F2], sc[:], AF.Exp)
                    nc.vector.tensor_tensor(
                        rhs2[:, :, 0:F2], M2[:, :, 0:F2],
                        rhs2[:, :, F2:N2].to_broadcast([128, nk, F2]), OP.mult,
                    )

                    for wb in range(nw):
                        w = b * G + wb
                        rows = WIN if w < NW - 1 else c.LASTROWS
                        pw2 = pwp.tile([128, N2], f32, tag="pw")
                        cl = [wb * KA + i for i in range(KA)] + [
                            nA + wb * KB + i for i in range(KB)
                        ]
                        for cidx, j in enumerate(cl):
                            S = sp.tile([128, 128], f32, tag="S")
                            nc.vector.tensor_tensor(
                                S[:], dl_t[:, j : j + 1].to_broadcast([128, 128]),
                                iotaf[:], OP.is_equal,
                            )
                            nc.tensor.matmul(
                                pw2[:], lhsT=S[:], rhs=rhs2[:, j, :],
                                start=(cidx == 0), stop=(cidx == len(cl) - 1),
                            )
                        # epilogue: normalize, +b2, log_softmax
                        rcp2 = winp.tile([128, 1], f32, tag="rcp")
                        nc.vector.reciprocal(rcp2[:], pw2[:, F2:N2])
                        o = winp.tile([128, F2], f32, tag="h1p")
                        nc.vector.tensor_tensor(
                            o[:], pw2[:, 0:F2], rcp2[:].to_broadcast([128, F2]), OP.mult
                        )
                        nc.vector.tensor_add(o[:], o[:], b2s[:])
                        mx = winp.tile([128, 1], f32, tag="mx")
                        nc.vector.reduce_max(mx[:], o[:], axis=AX.X)
                        nc.vector.tensor_scalar_mul(mx[:], mx[:], -1.0)
                        eo = winp.tile([128, F2], f32, tag="em")
                        se = winp.tile([128, 1], f32, tag="se")
                        nc.scalar.activation(eo[:], o[:], AF.Exp, bias=mx[:, 0:1], accum_out=se[:])
                        nc.scalar.activation(se[:], se[:], AF.Ln)
                        nc.vector.tensor_sub(mx[:], mx[:], se[:])
                        nc.vector.tensor_scalar_add(o[:], o[:], mx[:, 0:1])
                        nc.sync.dma_start(out_d[w * WIN : w * WIN + rows, :], o[:rows, :])

    return nc


def run(c, inputs, trace=False):
    ins = host_prep(c, **inputs)
    nc = build_program(c)
    nc.finalize()  # Bacc defers register allocation to finalize/compile
    res = run_bass_kernel_spmd(nc, ins, list(range(c.NC)), trace=trace)
    out = np.concatenate([res.results[i]["out"] for i in range(c.NC)], axis=0)
    return out, res


def kernel(**inputs) -> np.ndarray:
    out, _ = run(FULL, inputs)
    return out



# revision 4
# speedup vs baseline: 1.3476x; 1.3476x over previous
"""GATModel (2-layer GAT, N nodes, E edges + self-loops) on 8 Trainium2 NeuronCores.

Sharding: nodes and edges are sharded by destination-node owner (8 cores x
N/8 nodes). Per core, destination nodes are processed in 128-node windows;
edges of a window are padded into 128-edge chunks. Message gathers use
dma_gather (int16 indices -> the node table is split into an A half
[0, TSPLIT) and a B half [TSPLIT, N)); the segment softmax + scatter-add is
one-hot matmuls accumulated in PSUM (trailing columns carry the softmax
denominators). Layer-1 attention logits are linear in x and are precomputed
on host per edge. Between layers, each core's compact [h2|s2src|s2dst] rows
are AllGathered (4 overlapped chunks) and restrided into 512-byte rows for
the layer-2 gathers.
"""

import math

import numpy as np

import concourse.bacc as bacc
import concourse.bass as bass
import concourse.mybir as mybir
import concourse.tile as tile
from concourse.bass_utils import run_bass_kernel_spmd
from concourse.masks import make_identity

F32 = mybir.dt.float32
I16 = mybir.dt.int16
I32 = mybir.dt.int32
AF = mybir.ActivationFunctionType
OP = mybir.AluOpType
AX = mybir.AxisListType

NEG_SLOPE = 0.2


class Cfg:
    def __init__(self, n, in_dim, hid, heads1, out_dim, tsplit, g=3):
        self.N = n
        self.IN = in_dim          # 256
        self.C1 = hid             # 16
        self.H1 = heads1          # 8
        self.F1 = hid * heads1    # 128
        self.F2 = out_dim         # 64
        self.NC = 8
        assert n % self.NC == 0
        self.NLOC = n // self.NC
        self.WIN = 128
        self.NW = math.ceil(self.NLOC / self.WIN)
        self.LASTROWS = self.NLOC - (self.NW - 1) * self.WIN
        self.TSPLIT = tsplit
        self.G = g
        self.NB = math.ceil(self.NW / g)


FULL = Cfg(50000, 256, 16, 8, 64, 32768)


def _wrap_idx(ids):
    """int16 index list (len % 128 == 0) -> [128, L/16] dma_gather layout."""
    L = len(ids)
    assert L % 128 == 0
    w = ids.reshape(L // 16, 16).T
    return np.tile(w, (8, 1)).astype(np.int16)


def host_prep(c, x, edge_index, W1, a1_src, a1_dst, b1, W2, a2_src, a2_dst, b2):
    loop = np.arange(c.N, dtype=np.int64)
    src = np.concatenate([np.asarray(edge_index[0]), loop]).astype(np.int64)
    dst = np.concatenate([np.asarray(edge_index[1]), loop]).astype(np.int64)

    x = np.asarray(x, dtype=np.float32)
    W1 = np.asarray(W1, dtype=np.float32)
    W2 = np.asarray(W2, dtype=np.float32)

    # layer-1 logits are linear in x: s1_src = x @ (W1 . a1_src) etc.
    W1r = W1.reshape(c.IN, c.H1, c.C1)
    Wa1s = np.einsum("ihc,hc->ih", W1r, np.asarray(a1_src, np.float32))
    Wa1d = np.einsum("ihc,hc->ih", W1r, np.asarray(a1_dst, np.float32))
    ssum_e = (x @ Wa1s)[src] + (x @ Wa1d)[dst]  # [E, H1]

    owner = dst // c.NLOC
    KA = KB = 1
    percore = []
    for ci in range(c.NC):
        m = owner == ci
        s_c = src[m]
        dl_c = dst[m] - ci * c.NLOC
        ss_c = ssum_e[m]
        win = dl_c // c.WIN
        isB = s_c >= c.TSPLIT
        wins = []
        for w in range(c.NW):
            wm = win == w
            a_m = wm & ~isB
            b_m = wm & isB
            ea = (s_c[a_m], dl_c[a_m], ss_c[a_m])
            eb = (s_c[b_m] - c.TSPLIT, dl_c[b_m], ss_c[b_m])
            KA = max(KA, math.ceil(max(len(ea[0]), 1) / 128))
            KB = max(KB, math.ceil(max(len(eb[0]), 1) / 128))
            wins.append((ea, eb))
        percore.append(wins)

    c.KA, c.KB = KA, KB
    nk_w = KA + KB

    batches = []
    offC = offA = offB = 0
    for b in range(c.NB):
        nw = min(c.G, c.NW - b * c.G)
        batches.append((nw, offC, offA, offB))
        offC += nw * nk_w
        offA += nw * KA * 8
        offB += nw * KB * 8
    c.batches = batches
    c.TOTC, c.TOTA, c.TOTB = offC, offA, offB

    ins = []
    for ci in range(c.NC):
        wins = percore[ci]
        idxA = np.zeros((128, offA), np.int16)
        idxB = np.zeros((128, offB), np.int16)
        dstloc = np.full((128, offC), -1.0, np.float32)
        ssum = np.zeros((128, offC * c.H1), np.float32)
        dstid = np.zeros((128, offC), np.int32)

        for b, (nw, oc, oa, ob) in enumerate(batches):
            a_ids = np.zeros(nw * KA * 128, np.int16)
            b_ids = np.zeros(nw * KB * 128, np.int16)
            for wb in range(nw):
                w = b * c.G + wb
                for half, ids_arr, K, slot0, chunk0 in (
                    (0, a_ids, KA, wb * KA * 128, oc + wb * KA),
                    (1, b_ids, KB, wb * KB * 128, oc + nw * KA + wb * KB),
                ):
                    sH, dlH, ssH = wins[w][half]
                    n = len(sH)
                    assert n <= K * 128
                    ids_arr[slot0 : slot0 + n] = sH.astype(np.int16)
                    wcH = (dlH % c.WIN).astype(np.float32)
                    for jj in range(K):
                        lo, hi = jj * 128, min(n, (jj + 1) * 128)
                        if hi <= lo:
                            break
                        j = chunk0 + jj
                        dstloc[0 : hi - lo, j] = wcH[lo:hi]
                        dstid[0 : hi - lo, j] = dlH[lo:hi]
                        ssum[0 : hi - lo, j * c.H1 : (j + 1) * c.H1] = ssH[lo:hi]
            idxA[:, oa : oa + nw * KA * 8] = _wrap_idx(a_ids)
            idxB[:, ob : ob + nw * KB * 8] = _wrap_idx(b_ids)

        # s2dst gather indices: local dst node id per slot, in batch slot order
        idxD = np.zeros((128, offC * 8), np.int16)
        for b, (nw, oc, oa, ob) in enumerate(batches):
            nk = nw * nk_w
            ids = dstid[:, oc : oc + nk].T.reshape(-1).astype(np.int16)
            idxD[:, oc * 8 : (oc + nk) * 8] = _wrap_idx(ids)

        ins.append(dict(idxA=idxA, idxB=idxB, idxD=idxD, dstloc=dstloc, ssum=ssum))

    xT = np.ascontiguousarray(x.T)
    W2e = np.zeros((c.F1, 66), np.float32)
    W2e[:, : c.F2] = W2
    W2e[:, c.F2] = W2 @ np.asarray(a2_src, np.float32)[0]
    W2e[:, c.F2 + 1] = W2 @ np.asarray(a2_dst, np.float32)[0]
    b1t = np.tile(np.asarray(b1, np.float32)[None, :], (128, 1))
    b2t = np.tile(np.asarray(b2, np.float32)[None, :], (128, 1))
    for d in ins:
        d.update(xT=xT, W1=W1, W2e=W2e, b1t=b1t, b2t=b2t)
    return ins


def build_program(c, reps=1):
    nc = bacc.Bacc()
    f32 = F32
    H1, F1, F2, C1 = c.H1, c.F1, c.F2, c.C1
    KA, KB, G, WIN, NW = c.KA, c.KB, c.G, c.WIN, c.NW
    N1 = F1 + H1   # layer-1 rhs width (messages + denom cols)
    N2 = F2 + 1    # layer-2 rhs width

    xT = nc.dram_tensor("xT", [c.IN, c.N], f32, kind="ExternalInput")
    W1 = nc.dram_tensor("W1", [c.IN, F1], f32, kind="ExternalInput")
    W2e = nc.dram_tensor("W2e", [F1, 66], f32, kind="ExternalInput")
    b1t_d = nc.dram_tensor("b1t", [128, F1], f32, kind="ExternalInput")
    b2t_d = nc.dram_tensor("b2t", [128, F2], f32, kind="ExternalInput")
    idxA_d = nc.dram_tensor("idxA", [128, c.TOTA], I16, kind="ExternalInput")
    idxB_d = nc.dram_tensor("idxB", [128, c.TOTB], I16, kind="ExternalInput")
    dstloc_d = nc.dram_tensor("dstloc", [128, c.TOTC], f32, kind="ExternalInput")
    ssum_d = nc.dram_tensor("ssum", [128, c.TOTC * H1], f32, kind="ExternalInput")
    idxD_d = nc.dram_tensor("idxD", [128, c.TOTC * 8], I16, kind="ExternalInput")
    out_d = nc.dram_tensor("out", [c.NLOC, F2], f32, kind="ExternalOutput")

    table1 = nc.dram_tensor("table1", [c.N, F1], f32)
    table2 = nc.dram_tensor("table2", [c.N, 128], f32)
    agin = nc.dram_tensor("agin", [c.NLOC, 66], f32)
    # per-local-node s2dst, padded to 256B rows so dma_gather can fetch it
    s2dst_pad = nc.dram_tensor("s2dstpad", [c.NLOC, 64], f32)

    # AllGather chunks (by whole windows)
    nag = min(4, NW)
    per = math.ceil(NW / nag)
    wchunks = []
    w0 = 0
    while w0 < NW:
        wchunks.append((w0, min(w0 + per, NW)))
        w0 = min(w0 + per, NW)
    ag_rows = [(wl * WIN, min(wh * WIN, c.NLOC)) for wl, wh in wchunks]
    ag_out = [
        nc.dram_tensor(f"agout{k}", [c.NC * (rhi - rlo), 66], f32, addr_space="Shared")
        for k, (rlo, rhi) in enumerate(ag_rows)
    ]
    rg = [list(range(c.NC))]

    with tile.TileContext(nc) as tc:
        with (
            tc.tile_pool(name="const", bufs=1) as constp,
            tc.tile_pool(name="p1", bufs=3) as p1p,
            tc.tile_pool(name="meta", bufs=2) as metap,
            tc.tile_pool(name="big", bufs=2) as bigp,
            tc.tile_pool(name="schunk", bufs=6) as sp,
            tc.tile_pool(name="win", bufs=3) as winp,
            tc.tile_pool(name="pw", bufs=3, space="PSUM") as pwp,
            tc.tile_pool(name="ph", bufs=2, space="PSUM") as php,
            tc.tile_pool(name="ptp", bufs=2, space="PSUM") as ptp,
            tc.tile_pool(name="pp2", bufs=1, space="PSUM") as pp2,
        ):
            # ---- constants ----
            ident = constp.tile([128, 128], f32)
            make_identity(nc, ident[:])
            iota_i = constp.tile([128, 128], I32)
            nc.gpsimd.iota(iota_i[:], pattern=[[1, 128]], base=0, channel_multiplier=0)
            iotaf = constp.tile([128, 128], f32)
            nc.vector.tensor_copy(iotaf[:], iota_i[:])
            w1a = constp.tile([128, F1], f32)
            nc.sync.dma_start(w1a[:], W1[0:128, :])
            w1b = constp.tile([128, F1], f32)
            nc.sync.dma_start(w1b[:], W1[128:256, :])
            w2t = constp.tile([128, 66], f32)
            nc.sync.dma_start(w2t[:], W2e[:, :])
            b1s = constp.tile([128, F1], f32)
            nc.sync.dma_start(b1s[:], b1t_d[:, :])
            b2s = constp.tile([128, F2], f32)
            nc.sync.dma_start(b2s[:], b2t_d[:, :])

            for _rep in range(reps):
                # ---- P1: table1 = x @ W1 for all nodes (replicated) ----
                # 4 row-tiles (512 nodes) per DMA to stay off the sync queue.
                TB = 4
                nfull = c.N // (128 * TB)
                tails = []
                r = nfull * 128 * TB
                while r < c.N:
                    tails.append((r, min(128, c.N - r)))
                    r += 128
                for t in range(nfull + len(tails)):
                    if t < nfull:
                        r0, nt, last = t * 128 * TB, TB, 128
                    else:
                        r0, last = tails[t - nfull]
                        nt = 1
                    cols = (nt - 1) * 128 + last
                    xa = p1p.tile([128, 128 * TB], f32, tag="xa")
                    nc.sync.dma_start(xa[:, :cols], xT[0:128, r0 : r0 + cols])
                    xb = p1p.tile([128, 128 * TB], f32, tag="xb")
                    nc.sync.dma_start(xb[:, :cols], xT[128:256, r0 : r0 + cols])
                    hs = p1p.tile([128, TB, F1], f32, tag="hs")
                    for j in range(nt):
                        rows = 128 if j < nt - 1 else last
                        ph = php.tile([128, F1], f32, tag="ph")
                        nc.tensor.matmul(
                            ph[:rows, :], lhsT=xa[:, j * 128 : j * 128 + rows],
                            rhs=w1a[:], start=True, stop=False,
                        )
                        nc.tensor.matmul(
                            ph[:rows, :], lhsT=xb[:, j * 128 : j * 128 + rows],
                            rhs=w1b[:], start=False, stop=True,
                        )
                        nc.scalar.activation(hs[:rows, j, :], ph[:rows, :], AF.Copy)
                    if nt > 1:
                        nc.sync.dma_start(
                            table1[r0 : r0 + cols, :].rearrange(
                                "(t p) f -> p t f", p=128
                            ),
                            hs[:, :nt, :],
                        )
                    else:
                        nc.sync.dma_start(table1[r0 : r0 + last, :], hs[:last, 0, :])

                # ---- P2 (layer-1 edges) + fused P3 + chunked AllGather ----
                ag_issued = 0
                for b, (nw, oc, oa, ob) in enumerate(c.batches):
                    nA, nB_ = nw * KA, nw * KB
                    nk = nA + nB_
                    idxa_t = metap.tile([128, nA * 8], I16, tag="idxa")
                    nc.sync.dma_start(idxa_t[:], idxA_d[:, oa : oa + nA * 8])
                    idxb_t = metap.tile([128, nB_ * 8], I16, tag="idxb")
                    nc.sync.dma_start(idxb_t[:], idxB_d[:, ob : ob + nB_ * 8])
                    dl_t = metap.tile([128, nk], f32, tag="dl")
                    nc.sync.dma_start(dl_t[:], dstloc_d[:, oc : oc + nk])
                    ss_t = metap.tile([128, nk * H1], f32, tag="ss")
                    nc.sync.dma_start(ss_t[:], ssum_d[:, oc * H1 : (oc + nk) * H1])

                    M1 = bigp.tile([128, nk, 128], f32, tag="M")
                    nc.gpsimd.dma_gather(
                        M1[:, 0:nA, :], table1[0 : c.TSPLIT, :], idxa_t[:],
                        nA * 128, nA * 128, F1, single_packet=True,
                    )
                    nc.gpsimd.dma_gather(
                        M1[:, nA:nk, :], table1[c.TSPLIT : c.N, :], idxb_t[:],
                        nB_ * 128, nB_ * 128, F1, single_packet=True,
                    )

                    rhs = bigp.tile([128, nk, N1], f32, tag="rhs")
                    lr = metap.tile([128, nk * H1], f32, tag="lr")
                    nc.vector.tensor_scalar_mul(lr[:], ss_t[:], NEG_SLOPE)
                    nc.vector.tensor_tensor(lr[:], lr[:], ss_t[:], OP.max)
                    nc.scalar.activation(
                        rhs[:, :, F1:N1],
                        lr[:].rearrange("p (k h) -> p k h", h=H1),
                        AF.Exp,
                    )
                    nc.vector.tensor_tensor(
                        out=rhs[:, :, 0:F1].rearrange("p k (h ch) -> p k h ch", h=H1),
                        in0=M1[:, :, :].rearrange("p k (h ch) -> p k h ch", h=H1),
                        in1=rhs[:, :, F1:N1][:, :, :, None].to_broadcast([128, nk, H1, C1]),
                        op=OP.mult,
                    )

                    for wb in range(nw):
                        w = b * G + wb
                        rows = WIN if w < NW - 1 else c.LASTROWS
                        pw = pwp.tile([128, N1], f32, tag="pw")
                        cl = [wb * KA + i for i in range(KA)] + [
                            nA + wb * KB + i for i in range(KB)
                        ]
                        for cidx, j in enumerate(cl):
                            S = sp.tile([128, 128], f32, tag="S")
                            nc.vector.tensor_tensor(
                                S[:], dl_t[:, j : j + 1].to_broadcast([128, 128]),
                                iotaf[:], OP.is_equal,
                            )
                            nc.tensor.matmul(
                                pw[:], lhsT=S[:], rhs=rhs[:, j, :],
                                start=(cidx == 0), stop=(cidx == len(cl) - 1),
                            )
                        # window epilogue: normalize, +b1, ELU
                        rcp = winp.tile([128, H1], f32, tag="rcp")
                        nc.vector.reciprocal(rcp[:], pw[:, F1:N1])
                        h1p = winp.tile([128, F1], f32, tag="h1p")
                        nc.vector.tensor_tensor(
                            h1p[:].rearrange("p (h ch) -> p h ch", h=H1),
                            pw[:, 0:F1].rearrange("p (h ch) -> p h ch", h=H1),
                            rcp[:, :, None].to_broadcast([128, H1, C1]),
                            OP.mult,
                        )
                        nc.vector.tensor_add(h1p[:], h1p[:], b1s[:])
                        em = winp.tile([128, F1], f32, tag="em")
                        nc.vector.tensor_scalar_min(em[:], h1p[:], 0.0)
                        nc.scalar.activation(em[:], em[:], AF.Exp)
                        nc.vector.tensor_scalar_max(h1p[:], h1p[:], 0.0)
                        nc.vector.tensor_add(h1p[:], h1p[:], em[:])
                        nc.vector.tensor_scalar_add(h1p[:], h1p[:], -1.0)
                        # fused P3: [h2 | s2src | s2dst] for this window
                        tp = ptp.tile([128, 128], f32, tag="tp")
                        nc.tensor.transpose(tp[:], h1p[:], ident[:])
                        h1T = winp.tile([128, 128], f32, tag="h1T")
                        nc.vector.tensor_copy(h1T[:], tp[:])
                        p2 = pp2.tile([128, 66], f32, tag="p2")
                        nc.tensor.matmul(
                            p2[:rows, :], lhsT=h1T[:, :rows], rhs=w2t[:], start=True, stop=True
                        )
                        t2 = winp.tile([128, 66], f32, tag="t2")
                        nc.vector.tensor_copy(t2[:rows, :], p2[:rows, :])
                        nc.sync.dma_start(agin[w * WIN : w * WIN + rows, :], t2[:rows, :])
                        nc.sync.dma_start(
                            s2dst_pad[w * WIN : w * WIN + rows, 0:1], t2[:rows, 65:66]
                        )

                    while ag_issued < len(wchunks) and wchunks[ag_issued][1] <= (b + 1) * G:
                        k = ag_issued
                        rlo, rhi = ag_rows[k]
                        nc.gpsimd.collective_compute(
                            "AllGather", OP.bypass, replica_groups=rg,
                            ins=[agin[rlo:rhi, :]], outs=[ag_out[k][:, :]],
                        )
                        rk = rhi - rlo
                        for r in range(c.NC):
                            nc.sync.dma_start(
                                table2[r * c.NLOC + rlo : r * c.NLOC + rhi, 0:66],
                                ag_out[k][r * rk : (r + 1) * rk, :],
                            )
                        ag_issued += 1

                # ---- P4: layer-2 edges ----
                for b, (nw, oc, oa, ob) in enumerate(c.batches):
                    nA, nB_ = nw * KA, nw * KB
                    nk = nA + nB_
                    idxa_t = metap.tile([128, nA * 8], I16, tag="idxa")
                    nc.sync.dma_start(idxa_t[:], idxA_d[:, oa : oa + nA * 8])
                    idxb_t = metap.tile([128, nB_ * 8], I16, tag="idxb")
                    nc.sync.dma_start(idxb_t[:], idxB_d[:, ob : ob + nB_ * 8])
                    dl_t = metap.tile([128, nk], f32, tag="dl")
                    nc.sync.dma_start(dl_t[:], dstloc_d[:, oc : oc + nk])
                    idxd_t = metap.tile([128, nk * 8], I16, tag="idxd")
                    nc.sync.dma_start(idxd_t[:], idxD_d[:, oc * 8 : (oc + nk) * 8])

                    M2 = bigp.tile([128, nk, 128], f32, tag="M")
                    nc.gpsimd.dma_gather(
                        M2[:, 0:nA, :], table2[0 : c.TSPLIT, :], idxa_t[:],
                        nA * 128, nA * 128, 128, single_packet=True,
                    )
                    nc.gpsimd.dma_gather(
                        M2[:, nA:nk, :], table2[c.TSPLIT : c.N, :], idxb_t[:],
                        nB_ * 128, nB_ * 128, 128, single_packet=True,
                    )
                    SD = bigp.tile([128, nk, 64], f32, tag="SD")
                    nc.gpsimd.dma_gather(
                        SD[:, :, :], s2dst_pad[:, :], idxd_t[:], nk * 128, nk * 128, 64, single_packet=True
                    )
                    sc = metap.tile([128, nk], f32, tag="sc")
                    nc.vector.tensor_tensor(sc[:], M2[:, :, F2], SD[:, :, 0], OP.add)
                    sc2 = metap.tile([128, nk], f32, tag="sc2")
                    nc.vector.tensor_scalar_mul(sc2[:], sc[:], NEG_SLOPE)
                    nc.vector.tensor_tensor(sc[:], sc[:], sc2[:], OP.max)
                    rhs2 = bigp.tile([128, nk, N2], f32, tag="rhs")
                    nc.scalar.activation(rhs2[:, :, F2], sc[:], AF.Exp)
                    nc.vector.tensor_tensor(
                        rhs2[:, :, 0:F2], M2[:, :, 0:F2],
                        rhs2[:, :, F2:N2].to_broadcast([128, nk, F2]), OP.mult,
                    )

                    for wb in range(nw):
                        w = b * G + wb
                        rows = WIN if w < NW - 1 else c.LASTROWS
                        pw2 = pwp.tile([128, N2], f32, tag="pw")
                        cl = [wb * KA + i for i in range(KA)] + [
                            nA + wb * KB + i for i in range(KB)
                        ]
                        for cidx, j in enumerate(cl):
                            S = sp.tile([128, 128], f32, tag="S")
                            nc.vector.tensor_tensor(
                                S[:], dl_t[:, j : j + 1].to_broadcast([128, 128]),
                                iotaf[:], OP.is_equal,
                            )
                            nc.tensor.matmul(
                                pw2[:], lhsT=S[:], rhs=rhs2[:, j, :],
                                start=(cidx == 0), stop=(cidx == len(cl) - 1),
                            )
                        # epilogue: normalize, +b2, log_softmax
                        rcp2 = winp.tile([128, 1], f32, tag="rcp")
                        nc.vector.reciprocal(rcp2[:], pw2[:, F2:N2])
                        o = winp.tile([128, F2], f32, tag="h1p")
                        nc.vector.tensor_tensor(
                            o[:], pw2[:, 0:F2], rcp2[:].to_broadcast([128, F2]), OP.mult
                        )
                        nc.vector.tensor_add(o[:], o[:], b2s[:])
                        mx = winp.tile([128, 1], f32, tag="mx")
                        nc.vector.reduce_max(mx[:], o[:], axis=AX.X)
                        nc.vector.tensor_scalar_mul(mx[:], mx[:], -1.0)
                        eo = winp.tile([128, F2], f32, tag="em")
                        se = winp.tile([128, 1], f32, tag="se")
                        nc.scalar.activation(eo[:], o[:], AF.Exp, bias=mx[:, 0:1], accum_out=se[:])
                        nc.scalar.activation(se[:], se[:], AF.Ln)
                        nc.vector.tensor_sub(mx[:], mx[:], se[:])
                        nc.vector.tensor_scalar_add(o[:], o[:], mx[:, 0:1])
                        nc.sync.dma_start(out_d[w * WIN : w * WIN + rows, :], o[:rows, :])

    return nc


def run(c, inputs, trace=False):
    ins = host_prep(c, **inputs)
    nc = build_program(c)
    nc.finalize()  # Bacc defers register allocation to finalize/compile
    res = run_bass_kernel_spmd(nc, ins, list(range(c.NC)), trace=trace)
    out = np.concatenate([res.results[i]["out"] for i in range(c.NC)], axis=0)
    return out, res


def kernel(**inputs) -> np.ndarray:
    out, _ = run(FULL, inputs)
    return out

